# revision 8
# baseline (speedup 1.0000x reference)
"""MultiHeadCrossAttention Trainium2 kernel (8 NeuronCores, SPMD).

Sharding: core c -> (n = c // 2, g = c % 2). Each core handles one query
batch n and half the heads (8 of 16, embed slice g*512:(g+1)*512).

Host side: transpose queries/keys/values into [dim, tokens] layout (the
TensorEngine contracts along the partition dim, so both matmul operands
need the contraction dim on partitions), compact keys/values along KLEN
by the per-n mask (~50% survive), pad to KC = 128*T, cast to bf16.
The unnormalized AV outputs and softmax denominators come back per core;
the host divides while assembling/transposing the full output.

Device side per core (all matmuls bf16, fp32 PSUM accumulation):
  - qT/kT projections in transposed layout (lhsT = W chunk, rhs = xT);
    kT lands in per-head zero-padded slots (kTz) so the energy matmuls
    run with K=128 - full PE-array activity keeps the HAM clock at
    2.4 GHz (K=64 matmuls measurably re-throttle the PE to 1.2 GHz).
  - v projection in [k, emb] layout (lhsT = vT k-tile, rhs = W chunk).
  - energyT[k, q] = kTz.T @ qT per head, one PSUM bank per (head, k-tile).
  - exp on ScalarE (scale=1/8) PSUM->SBUF bf16. This is the kernel
    bottleneck: softmax exp runs at 1 elem/cycle/lane at 1.2 GHz and only
    ScalarE can do it, so the whole kernel is paced by the ACTIVATE
    stream. Items are (head, q-chunk, group of 3 k-tiles): one FD=1536
    ACTIVATE per item (96 total) amortizes the ~300-cycle per-ACTIVATE
    overhead that an FD=1024 split pays 160 times.
  - AV with lhsT = [v_h | valid-indicator | filler] (M=128) accumulated
    over k-tiles into one PSUM bank per (head, q-chunk): row 64 of the
    accumulator is the softmax denominator for free. Padded k rows have
    v=0 and indicator=0 so they contribute nothing anywhere.
  - PSUM budget (8 banks): energy 2 bufs x 3 banks + projections 1 +
    AV accumulator 1.
  - software pipeline: energy of item i+1 runs on the PE while exp of
    item i streams on ScalarE, kept alive across q-chunk AND head-pair
    boundaries; projection steps (incl. c=0's own) are injected into the
    item stream with data-deadlines so the first exp fires as soon as
    the first weight/q/k DMA chunks land (~10us instead of ~34us);
    inputs stream in column-chunks ordered by first use; junk matmuls
    during the initial DMA window pre-warm the PE clock gate; a dummy
    ACTIVATE preloads the exp table set (~2.7us) off the critical path.
"""

import math
import sys
from contextlib import ExitStack

import numpy as np

for _p in ("/opt/trn_rl_repo",):
    if _p not in sys.path:
        sys.path.insert(0, _p)

import ml_dtypes

import concourse.bass as bass  # noqa: F401  (import registers lowering deps)
import concourse.tile as tile
from concourse import bacc, mybir
from concourse.bass_utils import run_bass_kernel_spmd

BF16 = ml_dtypes.bfloat16

N, QLEN, KLEN = 4, 2048, 2048
QDIM = KVDIM = 512
EMBED, HEADS = 1024, 16
HEAD_DIM = 64
N_CORES = 8
QCH = 512  # q-chunk width (one PSUM bank of fp32)
SCALE = 1.0 / math.sqrt(HEAD_DIM)

_cache: dict = {}
last_exec_time_ns = None
last_results = None


def _build(T: int, ql: int = QLEN):
    """Build the per-core Bass program for KC = 128*T compacted kv tokens."""
    KC = 128 * T
    dt = mybir.dt
    nc = bacc.Bacc("TRN2", target_bir_lowering=False, debug=False)

    qT_d = nc.dram_tensor("qt", [QDIM, ql], dt.bfloat16, kind="ExternalInput").ap()
    kT_d = nc.dram_tensor("kt", [KVDIM, KC], dt.bfloat16, kind="ExternalInput").ap()
    vT_d = nc.dram_tensor("vt", [KVDIM, KC], dt.bfloat16, kind="ExternalInput").ap()
    wq_d = nc.dram_tensor("wq", [QDIM, 512], dt.bfloat16, kind="ExternalInput").ap()
    wk_d = nc.dram_tensor("wk", [KVDIM, 512], dt.bfloat16, kind="ExternalInput").ap()
    wv_d = nc.dram_tensor("wv", [KVDIM, 512], dt.bfloat16, kind="ExternalInput").ap()
    # per-row validity indicator (1.0 real kv token, 0.0 pad), [128, T]
    vind_d = nc.dram_tensor("vind", [128, T], dt.float32, kind="ExternalInput").ap()
    # rows (c*2+h)*65 .. +64: unnormalized AV.T ; row +64: denominator
    out_d = nc.dram_tensor("out", [520, ql], dt.float32, kind="ExternalOutput").ap()

    NQ = ql // QCH
    kcols = [(s, min(512, KC - s)) for s in range(0, KC, 512)]
    w_dram = {"wq": wq_d, "wk": wk_d, "wv": wv_d}
    # k-tile groups of 3 for batched exp (psE tile = 3 banks, FD=1536)
    groups = [tuple(range(t, min(t + 3, T))) for t in range(0, T, 3)]
    GW = 512 * min(3, T)

    with tile.TileContext(nc) as tc:
        with ExitStack() as ctx:
            persist = ctx.enter_context(tc.tile_pool(name="persist", bufs=1))

            qTin = [persist.tile([128, ql], dt.bfloat16, tag=f"qTin{j}", name=f"qTin{j}") for j in range(4)]
            kTin = [persist.tile([128, KC], dt.bfloat16, tag=f"kTin{j}", name=f"kTin{j}") for j in range(4)]
            vTin = [persist.tile([128, KC], dt.bfloat16, tag=f"vTin{j}", name=f"vTin{j}") for j in range(4)]
            wsb = {
                nm: [persist.tile([128, 512], dt.bfloat16, tag=f"{nm}{j}", name=f"{nm}{j}") for j in range(4)]
                for nm in ("wq", "wk", "wv")
            }
            qT = [persist.tile([128, ql], dt.bfloat16, tag=f"qT{c}", name=f"qT{c}") for c in range(4)]
            # kTz[c][:, h, :]: rows h*64..h*64+63 hold head h's kT rows, the
            # other 64 rows stay zero -> energy matmuls run with K=128 (full
            # PE array activity) at the same stream cost.
            kTz = [persist.tile([128, 2, KC], dt.bfloat16, tag=f"kTz{c}", name=f"kTz{c}") for c in range(4)]
            # [v_h (64) | indicator (1) | filler (63)]: M=128 keeps the full
            # array busy; output rows 65-127 are ignored.
            vsb = persist.tile([128, T, 4, 2, 128], dt.bfloat16, tag="v", name="v")
            vind = persist.tile([128, T], dt.float32, tag="vind", name="vind")
            junk = persist.tile([128, 512], dt.bfloat16, tag="junk", name="junk")
            dum = persist.tile([128, 1], dt.float32, tag="dum", name="dum")
            dumo = persist.tile([128, 1], dt.float32, tag="dumo", name="dumo")

            # ---- init: DVE memsets (fast) + exp-table preload ----
            nc.vector.memset(dum, 0.0)
            nc.scalar.activation(dumo, dum, mybir.ActivationFunctionType.Exp,
                                 scale=1.0)
            nc.vector.memset(junk, 1.0)
            nc.vector.memset(vsb, 1.0)
            for c in range(4):
                nc.vector.memset(kTz[c], 0.0)

            # ---- input DMA, ordered by first use, column-chunked.
            # c=0 only needs cols 0:128 of each weight; tails stream last.
            c0 = min(512, KC)
            for nm in ("wq", "wk"):
                for j in range(4):
                    nc.sync.dma_start(wsb[nm][j][:, 0:128],
                                      w_dram[nm][j * 128:(j + 1) * 128, 0:128])
            nc.sync.dma_start(vind, vind_d)
            for j in range(4):
                nc.sync.dma_start(qTin[j][:, 0:512], qT_d[j * 128:(j + 1) * 128, 0:512])
            for j in range(4):
                nc.sync.dma_start(kTin[j][:, 0:c0], kT_d[j * 128:(j + 1) * 128, 0:c0])
            for j in range(4):  # vproj does all 4 c at once -> full wv early
                nc.sync.dma_start(wsb["wv"][j], w_dram["wv"][j * 128:(j + 1) * 128, :])
            for j in range(4):
                nc.sync.dma_start(vTin[j][:, 0:c0], vT_d[j * 128:(j + 1) * 128, 0:c0])
            if KC > 512:
                for j in range(4):
                    nc.sync.dma_start(kTin[j][:, 512:KC], kT_d[j * 128:(j + 1) * 128, 512:KC])
                for j in range(4):
                    nc.sync.dma_start(vTin[j][:, 512:KC], vT_d[j * 128:(j + 1) * 128, 512:KC])
            for s in range(512, ql, 512):
                for j in range(4):
                    nc.sync.dma_start(qTin[j][:, s:s + 512], qT_d[j * 128:(j + 1) * 128, s:s + 512])
            for nm in ("wq", "wk"):
                for j in range(4):
                    nc.sync.dma_start(wsb[nm][j][:, 128:512],
                                      w_dram[nm][j * 128:(j + 1) * 128, 128:512])

            with tc.tile_pool(name="psA", bufs=1, space="PSUM") as psA, \
                 tc.tile_pool(name="psE", bufs=2, space="PSUM") as psE, \
                 tc.tile_pool(name="psO", bufs=1, space="PSUM") as psO, \
                 tc.tile_pool(name="sbx", bufs=4) as sbx, \
                 tc.tile_pool(name="sbo", bufs=3) as sbo:

                # PE warm-up during the DMA window (HAM un-throttle)
                ps = psA.tile([128, QCH], dt.float32, tag="pA", name="pA")
                for r in range(6):
                    nc.tensor.matmul(ps, lhsT=junk[:, :128], rhs=junk,
                                     start=(r == 0), stop=(r == 5))

                # ---- projection steps (closures, injected into the
                #      attention item stream) ----
                def qp(c, q0):
                    ps = psA.tile([128, QCH], dt.float32, tag="pA", name="pA")
                    for j in range(4):
                        nc.tensor.matmul(
                            ps,
                            lhsT=wsb["wq"][j][:, c * 128:(c + 1) * 128],
                            rhs=qTin[j][:, q0 * QCH:(q0 + 1) * QCH],
                            start=(j == 0), stop=(j == 3),
                        )
                    nc.vector.tensor_copy(qT[c][:, q0 * QCH:(q0 + 1) * QCH], ps)

                def kp(c, ki):
                    s, w = kcols[ki]
                    ps = psA.tile([128, QCH], dt.float32, tag="pA", name="pA")
                    for j in range(4):
                        nc.tensor.matmul(
                            ps[:, :w],
                            lhsT=wsb["wk"][j][:, c * 128:(c + 1) * 128],
                            rhs=kTin[j][:, s:s + w],
                            start=(j == 0), stop=(j == 3),
                        )
                    nc.vector.tensor_copy(kTz[c][0:64, 0, s:s + w], ps[0:64, :w])
                    nc.vector.tensor_copy(kTz[c][64:128, 1, s:s + w], ps[64:128, :w])

                def vp(t):
                    # one N=512 matmul projects k-tile t for ALL 4 c-chunks
                    ps = psA.tile([128, QCH], dt.float32, tag="pA", name="pA")
                    for j in range(4):
                        nc.tensor.matmul(
                            ps,
                            lhsT=vTin[j][:, t * 128:(t + 1) * 128],
                            rhs=wsb["wv"][j],
                            start=(j == 0), stop=(j == 3),
                        )
                    for c in range(4):
                        for h in range(2):
                            nc.vector.tensor_copy(
                                vsb[:, t, c, h, 0:64],
                                ps[:, c * 128 + h * 64:c * 128 + (h + 1) * 64])
                            nc.vector.tensor_copy(vsb[:, t, c, h, 64:65],
                                                  vind[:, t:t + 1])

                def proj_tasks(c):
                    # vproj is done once (all c) during c=0's stream
                    return ([lambda q0=q0: qp(c, q0) for q0 in range(NQ)]
                            + [lambda ki=ki: kp(c, ki) for ki in range(len(kcols))])

                # ---- attention pipeline, items = (h, q0, group) ----
                items = [(h, q0, gi) for h in range(2) for q0 in range(NQ)
                         for gi in range(len(groups))]
                NG = len(groups)

                def emit_energy(c, h, q0, gi):
                    grp = groups[gi]
                    eh = psE.tile([128, GW], dt.float32, tag="e", name="e")
                    for b, t in enumerate(grp):
                        nc.tensor.matmul(
                            eh[:, b * QCH:(b + 1) * QCH],
                            lhsT=kTz[c][:, h, t * 128:(t + 1) * 128],
                            rhs=qT[c][:, q0 * QCH:(q0 + 1) * QCH],
                            start=True, stop=True,
                        )
                    return eh

                def emit_exp_av(c, h, q0, gi, eh, av):
                    grp = groups[gi]
                    gw = len(grp) * QCH
                    ex = sbx.tile([128, GW], dt.bfloat16, tag="x", name="x")
                    nc.scalar.activation(
                        ex[:, :gw], eh[:, :gw],
                        mybir.ActivationFunctionType.Exp,
                        scale=SCALE,
                    )
                    for b, t in enumerate(grp):
                        nc.tensor.matmul(
                            av,
                            lhsT=vsb[:, t, c, h, :],
                            rhs=ex[:, b * QCH:(b + 1) * QCH],
                            start=(t == 0), stop=(t == T - 1),
                        )

                def emit_out(c, h, q0, av):
                    ot = sbo.tile([65, QCH], dt.float32, tag="ot", name="ot")
                    nc.vector.tensor_copy(ot, av[0:65, :])
                    nc.sync.dma_start(
                        out_d[(c * 2 + h) * 65:(c * 2 + h) * 65 + 65,
                              q0 * QCH:(q0 + 1) * QCH], ot)

                # c=0's own projections carry data deadlines (item index by
                # which they must be emitted); later cs' are paced evenly.
                def c0_deadline(kind, arg):
                    if kind == "q":  # needed by E(0, q0, 0)
                        return arg * NG
                    if kind == "k":  # chunk ki first used by E of the group
                        t_lo = kcols[arg][0] // 128   # holding its first tile
                        return min(gi for gi, grp in enumerate(groups)
                                   if t_lo in grp)
                    # "v": tile t needed by AV(group of t), emitted 1 item late
                    g = next(gi for gi, grp in enumerate(groups) if arg in grp)
                    return min(g + 1, len(items) - 1)

                sched: dict = {i: [] for i in range(len(items))}
                for kind, n_args in (("q", NQ), ("k", len(kcols))):
                    for a in range(n_args):
                        fn = {"q": qp, "k": kp}[kind]
                        sched[c0_deadline(kind, a)].append(
                            lambda a=a, fn=fn: fn(0, a))
                for t in range(T):
                    sched[c0_deadline("v", t)].append(lambda t=t: vp(t))

                av_cur = None

                def flush_prev(prev):
                    nonlocal av_cur
                    pc, ph, pq0, pgi, peh = prev
                    if pgi == 0:
                        av_cur = psO.tile([128, QCH], dt.float32, tag="av",
                                          name="av")
                    emit_exp_av(pc, ph, pq0, pgi, peh, av_cur)
                    if pgi == NG - 1:
                        emit_out(pc, ph, pq0, av_cur)

                prev = None
                for c in range(4):
                    tasks = proj_tasks(c + 1) if c < 3 else []
                    n_it = len(items)
                    # c=0: its own projections run in the first half (gated
                    # by input DMA); c=1's wait for the weight-tail DMAs.
                    s0 = n_it // 2 if c == 0 else 0
                    span = n_it - s0
                    for i, (h, q0, gi) in enumerate(items):
                        if c == 0:
                            for fn in sched[i]:
                                fn()
                        # evenly paced projections for c+1
                        if i >= s0:
                            p = i - s0
                            for fn in tasks[(p * len(tasks)) // span:
                                            ((p + 1) * len(tasks)) // span]:
                                fn()
                        eh = emit_energy(c, h, q0, gi)
                        if prev is not None:
                            flush_prev(prev)
                        prev = (c, h, q0, gi, eh)
                flush_prev(prev)

    nc.compile()
    return nc


def _prepare(queries, keys, values, mask):
    """Host-side sharding: transpose, compact kv by mask, validity tiles."""
    m = np.asarray(mask).reshape(N, KLEN) != 0
    idx = [np.nonzero(m[n])[0] for n in range(N)]
    cnts = [len(i) for i in idx]
    T = max(1, (max(cnts) + 127) // 128)
    KC = 128 * T

    kT_full = np.ascontiguousarray(np.asarray(keys, np.float32)[0].T)
    vT_full = np.ascontiguousarray(np.asarray(values, np.float32)[0].T)
    q32 = np.asarray(queries, np.float32)

    qT_n, kT_n, vT_n, vind_n = [], [], [], []
    for n in range(N):
        kt = np.zeros((KVDIM, KC), np.float32)
        vt = np.zeros((KVDIM, KC), np.float32)
        kt[:, :cnts[n]] = kT_full[:, idx[n]]
        vt[:, :cnts[n]] = vT_full[:, idx[n]]
        ind = (np.arange(KC) < cnts[n]).astype(np.float32)
        vind_n.append(np.ascontiguousarray(ind.reshape(T, 128).T))
        kT_n.append(kt.astype(BF16))
        vT_n.append(vt.astype(BF16))
        qT_n.append(np.ascontiguousarray(q32[n].T).astype(BF16))
    return T, qT_n, kT_n, vT_n, vind_n


def kernel(queries, keys, values, mask, Wq, Wk, Wv, _trace=False):
    global last_exec_time_ns, last_results
    T, qT_n, kT_n, vT_n, vind_n = _prepare(queries, keys, values, mask)

    w_g = {}
    for nm, W in (("wq", Wq), ("wk", Wk), ("wv", Wv)):
        W = np.asarray(W, np.float32)
        w_g[nm] = [np.ascontiguousarray(W[:, g * 512:(g + 1) * 512]).astype(BF16)
                   for g in range(2)]

    nc = _cache.get(T)
    if nc is None:
        nc = _cache.setdefault(T, _build(T))

    in_maps = []
    for core in range(N_CORES):
        n, g = core // 2, core % 2
        in_maps.append({
            "qt": qT_n[n], "kt": kT_n[n], "vt": vT_n[n],
            "wq": w_g["wq"][g], "wk": w_g["wk"][g], "wv": w_g["wv"][g],
            "vind": vind_n[n],
        })

    res = run_bass_kernel_spmd(nc, in_maps, core_ids=list(range(N_CORES)),
                               trace=bool(_trace))
    last_exec_time_ns = res.exec_time_ns
    last_results = res

    full = np.empty((N, QLEN, EMBED), np.float32)
    for core in range(N_CORES):
        n, g = core // 2, core % 2
        o = res.results[core]["out"].reshape(8, 65, QLEN)
        vals = o[:, :64, :] / o[:, 64:65, :]          # [8, 64, QLEN]
        full[n, :, g * 512:(g + 1) * 512] = (
            vals.transpose(2, 0, 1).reshape(QLEN, 512)
        )
    return full
